# revision 41
# baseline (speedup 1.0000x reference)
"""Self-contained Trainium2 Bass kernel for nn_Attention (B=8, N=1024, C=1024, H=16, D=64).

Sharding: data-parallel over batch B across the 8 NeuronCores (one batch element
per core, no collectives). Per-core program (bf16 matmuls, fp32 accumulate):

  - x is PE-transposed to xT [C, N]; weights are DMA'd in f32 and cast to bf16
    on-chip (GPSIMD), avoiding slow SWDGE casting DMAs.
  - qkT[c',n] (transposed q/k) is computed per head-pair and interleaved with
    the attention pipeline so the TensorEngine fills the gaps while the scalar
    engine (ACT) streams the softmax exps (the wall-clock bottleneck).
  - Scores are computed transposed, sT[k,q] = kT.T @ qT, with two heads packed
    onto the PE array via tile_position row groups. p = exp(sT*scale) on ACT
    (bf16, no max-subtraction: scores are O(5) so exp cannot overflow).
  - v carries an appended ones column (v_ext), so oT_ext = v_ext.T @ p also
    emits the softmax denominators as row D. Normalization: DVE reciprocal +
    DRAM-bounce partition-broadcast DMA + DVE multiply.
  - out[n, c'] = o_catT.T @ w_proj + bias.
"""

import numpy as np

B = 8
N = 1024          # tokens
C = 1024          # model dim
H = 16            # heads
D = 64            # head dim
SCALE = D ** -0.5
NT = N // 128     # token tiles
CT = C // 128     # channel tiles
HP = H // 2       # head pairs

_CACHE: dict = {}


def _build_program(repeat: int = 1, max_phase: int = 3):
    import concourse.mybir as mybir
    import concourse.tile as tile
    from concourse import bacc
    from concourse.masks import make_identity
    import concourse.bass as bass

    F32 = mybir.dt.float32
    BF16 = mybir.dt.bfloat16
    AF = mybir.ActivationFunctionType

    nc = bacc.Bacc("TRN2", target_bir_lowering=False, debug=False, num_devices=B)

    x_ext = nc.declare_dram_parameter("x", [N, C], F32, isOutput=False)
    wqkv_ext = nc.declare_dram_parameter("w_qkv", [C, 3 * C], F32, isOutput=False)
    bqkv_ext = nc.declare_dram_parameter("b_qkv", [3 * C], F32, isOutput=False)
    wproj_ext = nc.declare_dram_parameter("w_proj", [C, C], F32, isOutput=False)
    bproj_ext = nc.declare_dram_parameter("b_proj", [C], F32, isOutput=False)
    out_ext = nc.declare_dram_parameter("out", [N, C], F32, isOutput=True)

    x_ap = x_ext.ap()
    wqkv_ap = wqkv_ext.ap()
    bqkv_ap = bqkv_ext.ap()
    wproj_ap = wproj_ext.ap()
    bproj_ap = bproj_ext.ap()
    out_ap = out_ext.ap()

    def bcast_part(src_ap, parts):
        return bass.AP(
            tensor=src_ap.tensor,
            offset=src_ap.offset,
            ap=[[0, parts]] + [list(p) for p in src_ap.ap[1:]],
        )

    def bcast_row(src_1d_ap, parts):
        return bass.AP(
            tensor=src_1d_ap.tensor,
            offset=src_1d_ap.offset,
            ap=[[0, parts]] + [list(p) for p in src_1d_ap.ap],
        )

    with tile.TileContext(nc) as tc:
        # ---- persistent SBUF ----
        identity, _free_id = tc.tile([128, 128], F32, name="identity")
        make_identity(nc, identity)

        v_ext, _free_vext = tc.tile([128, NT, H, D + 1], BF16, name="v_ext")
        nc.vector.memset(v_ext[:, :, :, D : D + 1], 1.0)
        bq_pp, _free_bq = tc.tile([128, 2 * CT], F32, name="bq_pp")
        bv_bc, _free_bv = tc.tile([128, C], F32, name="bv_bc")
        bp_bc, _free_bp = tc.tile([128, C], F32, name="bp_bc")

        nc.sync.dma_start(
            out=bq_pp, in_=bqkv_ap[0 : 2 * C].rearrange("(t p) -> p t", p=128)
        )
        nc.sync.dma_start(out=bv_bc, in_=bcast_row(bqkv_ap[2 * C : 3 * C], 128))
        nc.sync.dma_start(out=bp_bc, in_=bcast_row(bproj_ap, 128))

        for rep in range(repeat):
            s = f"r{rep}_"

            o_catT, free_ocat = tc.tile([128, CT, N], BF16, name=s + "o_catT")
            wproj, free_wproj = tc.tile([128, CT, C], BF16, name=s + "wproj")
            xT, free_xT = tc.tile([128, CT, N], BF16, name=s + "xT")
            wv_bf, free_wv = tc.tile([128, CT, C], BF16, name=s + "wv_bf")

            wqkv_t = wqkv_ap.rearrange("(kt p) c -> p kt c", p=128)

            # ================= phase 0: x -> xT =================
            with (
                tc.tile_pool(name=s + "x_pool", bufs=1) as x_pool,
                tc.tile_pool(name=s + "pt_pool", bufs=4, space="PSUM") as pt_pool,
            ):
                x_all = x_pool.tile(
                    [128, NT, C], F32, name=f"{s}x_all", tag="x_all"
                )
                x_src = x_ap.rearrange("(i p) c -> p i c", p=128)
                HN = NT // 2
                for half in range(2):
                    isl = slice(half * HN, (half + 1) * HN)
                    nc.sync.dma_start(out=x_all[:, isl, :], in_=x_src[:, isl, :])
                    for j in range(CT):
                        for i in range(half * HN, (half + 1) * HN):
                            ps_t = pt_pool.tile(
                                [128, 128], F32, name=f"{s}ps_t{i}_{j}", tag="ps_t"
                            )
                            nc.tensor.transpose(
                                ps_t, x_all[:, i, j * 128 : (j + 1) * 128], identity
                            )
                            nc.vector.tensor_copy(
                                out=xT[:, j, i * 128 : (i + 1) * 128], in_=ps_t
                            )

            if max_phase == 0:
                nc.gpsimd.dma_start(out=out_ap[0:128, :], in_=xT[:, 0, :])
                free_wv()
                free_xT()
                free_wproj()
                free_ocat()
                continue

            # ---- interleaved main body ----
            with (
                tc.tile_pool(name=s + "wqk_stage", bufs=1) as wqk_stage_pool,
                tc.tile_pool(name=s + "wqk_pool", bufs=3) as wqk_pool,
                tc.tile_pool(name=s + "wv_stage", bufs=1) as wv_stage_pool,
                tc.tile_pool(name=s + "qk_pool", bufs=6) as qk_pool,
                tc.tile_pool(name=s + "ps_big", bufs=2, space="PSUM") as ps_big_pool,
                tc.tile_pool(name=s + "ps_o", bufs=1, space="PSUM") as ps_o_pool,
                tc.tile_pool(name=s + "ps_v", bufs=2, space="PSUM") as ps_v_pool,
                tc.tile_pool(name=s + "pT_pool", bufs=16) as pT_pool,
                tc.tile_pool(name=s + "l_pool", bufs=2) as l_pool,
                tc.tile_pool(name=s + "l_dram", bufs=2, space="DRAM") as l_dram_pool,
                tc.tile_pool(name=s + "wpstage", bufs=2) as wpstage_pool,
            ):

                def emit_wqk_strip(pj):
                    """Load + cast the two [C, 128] weight column strips for pair pj."""
                    stage = wqk_stage_pool.tile(
                        [128, CT, 256], F32, name=f"{s}wqs{pj}", tag="wqk_stage"
                    )
                    nc.scalar.dma_start(
                        out=stage[:, :, 0:128],
                        in_=wqkv_t[:, :, pj * 128 : (pj + 1) * 128],
                    )
                    nc.scalar.dma_start(
                        out=stage[:, :, 128:256],
                        in_=wqkv_t[:, :, C + pj * 128 : C + (pj + 1) * 128],
                    )
                    strip = wqk_pool.tile(
                        [128, CT, 256], BF16, name=f"{s}wqk{pj}", tag="wqk"
                    )
                    nc.gpsimd.tensor_copy(out=strip, in_=stage)
                    return strip

                def emit_wv_chunk(ch):
                    csl = slice(2 * C + ch * 512, 2 * C + ch * 512 + 512)
                    stage = wv_stage_pool.tile(
                        [128, CT, 512], F32, name=f"{s}wvs{ch}", tag="wv_stage"
                    )
                    nc.scalar.dma_start(out=stage, in_=wqkv_t[:, :, csl])
                    nc.gpsimd.tensor_copy(
                        out=wv_bf[:, :, ch * 512 : ch * 512 + 512], in_=stage
                    )

                def emit_qk_pair(pj, strip):
                    """qT/kT for head pair pj, chunk-major so the pair's first
                    score matmuls unblock after half the projection work."""
                    tiles = [
                        qk_pool.tile([128, N], BF16, name=f"{s}qk{pj}_{qk}", tag="qk")
                        for qk in range(2)
                    ]
                    for ch in range(2):
                        nsl = slice(ch * 512, ch * 512 + 512)
                        for qk in range(2):  # 0 = q, 1 = k
                            jj = qk * CT + pj
                            ps1 = ps_v_pool.tile(
                                [128, 512], F32, name=f"{s}ps1_{jj}_{ch}", tag="ps_v"
                            )
                            for kt in range(CT):
                                nc.tensor.matmul(
                                    ps1,
                                    strip[:, kt, qk * 128 : qk * 128 + 128],
                                    xT[:, kt, nsl],
                                    start=(kt == 0),
                                    stop=(kt == CT - 1),
                                )
                            nc.vector.tensor_scalar_add(
                                out=tiles[qk][:, nsl], in0=ps1,
                                scalar1=bq_pp[:, jj : jj + 1],
                            )
                    return tiles

                def emit_v_chunk(ch):
                    for m in range(NT):
                        ps_v = ps_v_pool.tile(
                            [128, 512], F32, name=f"{s}ps_vv{m}_{ch}", tag="ps_v"
                        )
                        for kt in range(CT):
                            nc.tensor.matmul(
                                ps_v,
                                xT[:, kt, m * 128 : (m + 1) * 128],
                                wv_bf[:, kt, ch * 512 : ch * 512 + 512],
                                start=(kt == 0),
                                stop=(kt == CT - 1),
                            )
                        nc.vector.tensor_add(
                            out=v_ext[:, m, ch * 8 : ch * 8 + 8, 0:D],
                            in0=ps_v.rearrange("p (h d) -> p h d", d=D),
                            in1=bv_bc[:, ch * 512 : ch * 512 + 512].rearrange(
                                "p (h d) -> p h d", d=D
                            ),
                        )

                def emit_attn_head(h, qp, kp):
                    pj, hh = h // 2, h % 2
                    hb = hh * 64
                    ps_o = ps_o_pool.tile(
                        [D + 1, N], F32, name=f"{s}ps_o{h}", tag="ps_o"
                    )
                    for kt in range(NT):
                        ksl = slice(kt * 128, (kt + 1) * 128)
                        ps_sc = ps_big_pool.tile(
                            [128, N], F32, name=f"{s}ps_sc{h}_{kt}", tag="ps_big"
                        )
                        for ch in range(2):
                            nsl = slice(ch * 512, ch * 512 + 512)
                            nc.tensor.matmul(
                                ps_sc[:, nsl],
                                kp[hb : hb + 64, ksl],
                                qp[hb : hb + 64, nsl],
                                start=True,
                                stop=True,
                                tile_position=(hb, 0),
                            )
                        pT = pT_pool.tile(
                            [128, N], BF16, name=f"{s}pT{h}_{kt}", tag="pT"
                        )
                        nc.scalar.activation(
                            out=pT, in_=ps_sc, func=AF.Exp, scale=SCALE
                        )
                        for ch in range(2):
                            nsl = slice(ch * 512, ch * 512 + 512)
                            nc.tensor.matmul(
                                ps_o[:, nsl],
                                v_ext[:, kt, h, :],
                                pT[:, nsl],
                                start=(kt == 0),
                                stop=(kt == NT - 1),
                            )
                    # drain PSUM fast (frees the bank for the next head's pv),
                    # then normalize from SBUF off the critical path
                    o_raw = l_pool.tile(
                        [D + 1, N], F32, name=f"{s}o_raw{h}", tag="o_raw", bufs=2
                    )
                    nc.vector.tensor_copy(out=o_raw, in_=ps_o)
                    l_inv = l_pool.tile([1, N], F32, name=f"{s}l_inv{h}", tag="l_inv")
                    nc.vector.reciprocal(out=l_inv, in_=o_raw[D : D + 1, :])
                    l_bc = l_pool.tile([D, N], F32, name=f"{s}l_bc{h}", tag="l_bc")
                    nc.gpsimd.partition_broadcast(l_bc, l_inv)
                    nc.vector.tensor_mul(
                        out=o_catT[hb : hb + 64, pj, :],
                        in0=o_raw[0:D, :],
                        in1=l_bc,
                    )

                # prologue: first pair's qk + v for heads 0..7
                strip0 = emit_wqk_strip(0)
                emit_wv_chunk(0)
                pair_tiles = emit_qk_pair(0, strip0)
                strip_next = emit_wqk_strip(1)
                emit_v_chunk(0)
                for h in range(H):
                    pj = h // 2
                    emit_attn_head(h, *pair_tiles)
                    if h % 2 == 0 and pj + 1 < HP:
                        next_tiles = emit_qk_pair(pj + 1, strip_next)
                        if pj + 2 < HP:
                            strip_next = emit_wqk_strip(pj + 2)
                    if h % 2 == 1:
                        pair_tiles = next_tiles
                    if h == 0:
                        emit_wv_chunk(1)
                        emit_v_chunk(1)
                    if h == 2:
                        # stream w_proj in during attention (cast on GPSIMD)
                        for kt in range(CT):
                            wp_stage = wpstage_pool.tile(
                                [128, C], F32, name=f"{s}wp_stage{kt}", tag="wp_stage"
                            )
                            nc.scalar.dma_start(
                                out=wp_stage,
                                in_=wproj_ap[kt * 128 : (kt + 1) * 128, :],
                            )
                            nc.gpsimd.tensor_copy(out=wproj[:, kt, :], in_=wp_stage)

            free_wv()
            free_xT()

            # ================= projection =================
            with (
                tc.tile_pool(name=s + "ps_y", bufs=4, space="PSUM") as ps_y_pool,
                tc.tile_pool(name=s + "y_pool", bufs=2) as y_pool,
            ):
                for m in range(NT):
                    y_sb = y_pool.tile([128, C], F32, name=f"{s}y_sb{m}", tag="y_sb")
                    for ch in range(2):
                        nsl = slice(ch * 512, ch * 512 + 512)
                        ps_y = ps_y_pool.tile(
                            [128, 512], F32, name=f"{s}ps_y{m}_{ch}", tag="ps_y"
                        )
                        for j in range(CT):
                            nc.tensor.matmul(
                                ps_y,
                                o_catT[:, j, m * 128 : (m + 1) * 128],
                                wproj[:, j, nsl],
                                start=(j == 0),
                                stop=(j == CT - 1),
                            )
                        nc.vector.tensor_add(
                            out=y_sb[:, nsl], in0=ps_y, in1=bp_bc[:, nsl]
                        )
                    nc.sync.dma_start(
                        out=out_ap[m * 128 : (m + 1) * 128, :], in_=y_sb
                    )

            free_wproj()
            free_ocat()

        _free_bp()
        _free_bv()
        _free_bq()
        _free_vext()
        _free_id()

    nc.compile()
    return nc


def get_program(repeat: int = 1, max_phase: int = 3):
    key = ("nc", repeat, max_phase)
    if key not in _CACHE:
        _CACHE[key] = _build_program(repeat, max_phase)
    return _CACHE[key]


def _get_runner():
    """Persistent jitted SPMD executor (avoids re-tracing per kernel() call).

    Mirrors concourse.bass2jax.run_bass_via_pjrt's multi-core path, but caches
    the compiled callable so repeat invocations cost only dispatch + transfer,
    and device-caches the (usually unchanged) weight arrays by content hash.
    """
    if "runner" in _CACHE:
        return _CACHE["runner"]

    import jax
    from jax.sharding import Mesh, PartitionSpec
    from jax.experimental.shard_map import shard_map
    import concourse.mybir as mybir
    from concourse.bass2jax import (
        _bass_exec_p,
        install_neuronx_cc_hook,
        partition_id_tensor,
    )

    nc = get_program()
    install_neuronx_cc_hook()
    partition_name = nc.partition_id_tensor.name if nc.partition_id_tensor else None

    in_names, out_names, out_avals, zero_outs = [], [], [], []
    for alloc in nc.m.functions[0].allocations:
        if not isinstance(alloc, mybir.MemoryLocationSet):
            continue
        name = alloc.memorylocations[0].name
        if alloc.kind == "ExternalInput":
            if name != partition_name:
                in_names.append(name)
        elif alloc.kind == "ExternalOutput":
            shape = tuple(alloc.tensor_shape)
            dtype = mybir.dt.np(alloc.dtype)
            out_names.append(name)
            out_avals.append(jax.core.ShapedArray(shape, dtype))
            zero_outs.append(np.zeros((B * shape[0], *shape[1:]), dtype))
    n_params = len(in_names)
    in_names_all = list(in_names) + list(out_names)
    if partition_name is not None:
        in_names_all.append(partition_name)

    def _body(*args):
        operands = list(args)
        if partition_name is not None:
            operands.append(partition_id_tensor())
        return tuple(
            _bass_exec_p.bind(
                *operands,
                out_avals=tuple(out_avals),
                in_names=tuple(in_names_all),
                out_names=tuple(out_names),
                lowering_input_output_aliases=(),
                sim_require_finite=True,
                sim_require_nnan=True,
                nc=nc,
            )
        )

    devices = jax.devices()[:B]
    mesh = Mesh(np.asarray(devices), ("core",))
    n_outs = len(out_avals)
    sharded = jax.jit(
        shard_map(
            _body,
            mesh=mesh,
            in_specs=(PartitionSpec("core"),) * (n_params + n_outs),
            out_specs=(PartitionSpec("core"),) * n_outs,
            check_rep=False,
        ),
        keep_unused=True,
    )

    sharding = jax.sharding.NamedSharding(mesh, PartitionSpec("core"))
    dev_cache: dict = {}

    def _to_device(name, concat):
        """Device-put with content-hash caching (weights repeat across calls)."""
        import hashlib

        digest = hashlib.blake2b(concat.tobytes(), digest_size=16).digest()
        hit = dev_cache.get(name)
        if hit is not None and hit[0] == digest:
            return hit[1]
        arr = jax.device_put(concat, sharding)
        dev_cache[name] = (digest, arr)
        return arr

    def run(in_maps):
        concat_in = [
            _to_device(
                name,
                np.concatenate([np.asarray(m[name]) for m in in_maps], axis=0),
            )
            for name in in_names
        ]
        outs = sharded(*concat_in, *zero_outs)
        return {
            name: np.asarray(outs[i]).reshape(B, *out_avals[i].shape)
            for i, name in enumerate(out_names)
        }

    _CACHE["runner"] = run
    return run


def kernel(x, w_qkv, b_qkv, w_proj, b_proj):
    x = np.ascontiguousarray(np.asarray(x, dtype=np.float32))
    shared = {
        "w_qkv": np.ascontiguousarray(np.asarray(w_qkv, dtype=np.float32)),
        "b_qkv": np.ascontiguousarray(np.asarray(b_qkv, dtype=np.float32)),
        "w_proj": np.ascontiguousarray(np.asarray(w_proj, dtype=np.float32)),
        "b_proj": np.ascontiguousarray(np.asarray(b_proj, dtype=np.float32)),
    }
    in_maps = [{"x": x[b], **shared} for b in range(B)]
    run = _get_runner()
    res = run(in_maps)
    return res["out"].astype(np.float32)
